# revision 25
# baseline (speedup 1.0000x reference)
"""Trainium2 Bass kernel for nn_DiffusionLoss (B=4, N=2048).

Decomposition
-------------
loss = align_term + bond_term, pooled over the whole batch, then scaled by
the per-sample ht factor.

* align term + all O(N) reductions (means, 3x3 Kabsch matrix, SVD, rotated
  residual norms) are tiny -> host numpy in f64.
* bond term: sum_ij w_i w_j (dp_ij - dg_ij)^2
    = sum_ij w_i w_j dp^2 + sum_ij w_i w_j dg^2 - 2 * sum_ij w_i w_j dp dg.
  The two squared terms expand analytically to O(N) sums (host, f64, exact).
  Only the cross term P = sum_ij w_i w_j dp_ij dg_ij needs the full N x N
  pairwise pass -> device.

Device trick: with augmented 5-vectors
  vp_i = [-2 xp_i, 1, |xp_i|^2],  up_j = w_j^2 [xp_j, |xp_j|^2, 1]
(and likewise vg/ug unscaled for the ground truth), the 25-dim outer
products vp_i (x) vg_i and up_j (x) ug_j satisfy
  (vp_i (x) vg_i) . (up_j (x) ug_j) = (w_j^2 d2p_ij) * (d2g_ij),
so ONE matmul (float32r: the PE's fast relaxed-fp32 path, 4x the strict
fp32 rate) produces w_j^2 d2p d2g per [128,512] tile. t_ij = w_j dp dg is
symmetric under (i,j) exchange once the host applies w_i, so only the
strictly-upper triangle is computed (block-triangular at 128-row x
512-col granularity, 0.625x the full work) and P[b] = 2 * device sum.

Pipeline per 128-row slot: PE writes the row-block's surviving j-chunks
into a 4-bank PSUM tile; DVE clamps f32r rounding noise to zero and, on
the diagonal chunk, multiplies the strictly-upper 0/1 mask in the same
pass (scalar_tensor_tensor: (m max 0) * mask); ACT computes sqrt fused
with the free-axis row-sum (accum_out). The per-row w_i factor and the
f64 reduction happen on the host. No collectives.

Sharding: one program for all 8 cores. Core c -> batch c//2; parity c%2
picks 8 of the batch's 16 row blocks, two from each width class
(512/1024/1536/2048 surviving columns), slotted in the fixed order
[512, 2048, 1024, 1536, 2048, 512, 1536, 1024] (narrow first and last
so the pipeline fills/drains fast). The diagonal-mask variant needed at
slot s is always at input position s%4 — the host rolls the 4 mask
variants per core parity.

Raw Bass (no TileContext): this walrus build allows only ~2 sync
commands per instruction, so all waits are standalone wait_ge
instructions and each compute instruction carries at most one sem
update. Input DMAs are split across HWDGE queues to run in parallel.
"""

import numpy as np

import concourse.bass as bass
from concourse import mybir
from concourse.bass_utils import run_bass_kernel_spmd

B = 4
N = 2048
NSLOT = 8
SIGMA_DATA = 16.0

F32 = mybir.dt.float32

# slot -> row-block bi, per core parity. Fixed width order for both:
# widths [512, 2048, 1024, 1536, 2048, 1024, 1536, 512] (narrow first so
# the pipeline fills fast, narrow last so it drains fast);
# jc(s) = bi//4 identical across parities;
# mask variant bi%4 = [0,1,2,3]*2 (parity 0) / [2,3,0,1]*2 (parity 1).
SLOTS_BY_PARITY = {
    0: [12, 1, 10, 7, 0, 9, 6, 15],
    1: [14, 3, 8, 5, 2, 11, 4, 13],
}
JC = [3, 0, 2, 1, 0, 2, 1, 3]

_NC_CACHE = None


def _build_nc():
    nmm = [4 - jc for jc in JC]
    ndve = [1 + (1 if jc < 3 else 0) for jc in JC]
    M = np.cumsum(nmm).tolist()              # pe_sem value after slot s
    D = np.cumsum(ndve).tolist()             # dve_sem value after slot s

    nc = bass.Bass("TRN2", target_bir_lowering=False, debug=False, num_devices=8)

    F32R = mybir.dt.float32r
    uv = nc.declare_dram_parameter("uv", [26, N + NSLOT * 128], F32R, isOutput=False)
    masks = nc.declare_dram_parameter("masks", [128, 4 * 512], F32, isOutput=False)
    res = nc.declare_dram_parameter("res", [128, NSLOT], F32, isOutput=True)

    with (
        nc.sbuf_tensor([26, N + NSLOT * 128], F32R) as uv_t,
        nc.sbuf_tensor([128, 4 * 512], F32) as masks_t,
        nc.sbuf_tensor([128, NSLOT], F32) as res_t,
        nc.sbuf_tensor([128, 4 * 512], F32) as mc0,
        nc.sbuf_tensor([128, 4 * 512], F32) as mc1,
        nc.sbuf_tensor([128, 4 * 512], F32) as tt,
        nc.sbuf_tensor([128, 1], F32) as bias_t,
        nc.psum_tensor([128, 4 * 512], F32) as psum0,
        nc.psum_tensor([128, 4 * 512], F32) as psum1,
        nc.semaphore("dma_in_sem") as dma_in_sem,
        nc.semaphore("dma_in2_sem") as dma_in2_sem,
        nc.semaphore("mask_sem") as mask_sem,
        nc.semaphore("mask2_sem") as mask2_sem,
        nc.semaphore("pe_sem") as pe_sem,
        nc.semaphore("dve_sem") as dve_sem,
        nc.semaphore("act_sem") as act_sem,
        nc.semaphore("dma_out_sem") as dma_out_sem,
        nc.semaphore("init_sem") as init_sem,
        nc.Block() as block,
    ):
        psums = [psum0, psum1]
        mcs = [mc0, mc1]
        UVW = N + NSLOT * 128

        @block.sync
        def _(sync):
            # input DMAs are spread across engines so the transfers (and
            # their fixed DGE setup costs) run in parallel
            sync.dma_start(out=uv_t[:, : UVW // 2], in_=uv[:, : UVW // 2]).then_inc(
                dma_in_sem, 16
            )
            sync.wait_ge(dma_out_sem, 16)

        @block.gpsimd
        def _(gp):
            gp.dma_start(
                out=uv_t[:, UVW // 2 :], in_=uv[:, UVW // 2 :]
            ).then_inc(dma_in2_sem, 16)
            gp.dma_start(
                out=masks_t[:, 2 * 512 :], in_=masks[:, 2 * 512 :]
            ).then_inc(mask2_sem, 16)

        @block.tensor
        def _(tensor):
            tensor.wait_ge(dma_in_sem, 16)
            tensor.wait_ge(dma_in2_sem, 16)
            for s in range(NSLOT):
                jc = JC[s]
                lhsT = uv_t[:, N + s * 128 : N + (s + 1) * 128]
                if s >= 2:
                    # psum slot s%2 fully consumed by slot s-2's DVE ops
                    tensor.wait_ge(dve_sem, D[s - 2])
                pt = psums[s % 2]
                for ch in range(jc, 4):
                    nc.tensor.matmul(
                        pt[:, ch * 512 : (ch + 1) * 512],
                        lhsT,
                        uv_t[:, ch * 512 : (ch + 1) * 512],
                        start=True,
                        stop=True,
                    ).then_inc(pe_sem, 1)

        @block.vector
        def _(vector):
            # mask positions 0-1 arrive via the ACT-issued DMA, 2-3 via
            # the Pool-issued one (needed from slot 2 onward).
            vector.wait_ge(mask_sem, 16)
            for s in range(NSLOT):
                jc, v = JC[s], s % 4
                if v == 2 and s < 4:
                    vector.wait_ge(mask2_sem, 16)
                vector.wait_ge(pe_sem, M[s])
                if s >= 2:
                    # mc slot s%2 last read by act(s-2)
                    vector.wait_ge(act_sem, s - 1)
                # diagonal chunk: clamp and apply strictly-upper mask
                nc.vector.scalar_tensor_tensor(
                    out=mcs[s % 2][:, jc * 512 : (jc + 1) * 512],
                    in0=psums[s % 2][:, jc * 512 : (jc + 1) * 512],
                    scalar=0.0,
                    in1=masks_t[:, v * 512 : (v + 1) * 512],
                    op0=mybir.AluOpType.max,
                    op1=mybir.AluOpType.mult,
                ).then_inc(dve_sem, 1)
                if jc < 3:
                    # remaining full chunks: clamp only
                    nc.vector.tensor_scalar_max(
                        mcs[s % 2][:, (jc + 1) * 512 :].rearrange(
                            "p (a b) -> p a b", b=512
                        ),
                        psums[s % 2][:, (jc + 1) * 512 :].rearrange(
                            "p (a b) -> p a b", b=512
                        ),
                        0.0,
                    ).then_inc(dve_sem, 1)

        @block.scalar
        def _(scalar):
            # ACT is idle at start: issue the first-half mask DMA from here
            scalar.dma_start(
                out=masks_t[:, : 2 * 512], in_=masks[:, : 2 * 512]
            ).then_inc(mask_sem, 16)
            # Same-engine RAW through SBUF is not pipeline-interlocked:
            # drain the memzero via a sem self-wait before the first read.
            nc.scalar.memzero(bias_t[:, :]).then_inc(init_sem, 1)
            scalar.wait_ge(init_sem, 1)
            for s in range(NSLOT):
                jc = JC[s]
                scalar.wait_ge(dve_sem, D[s])
                if s > 0:
                    # drain previous activation's tt write (same-engine WAW)
                    scalar.wait_ge(act_sem, s)
                nc.scalar.activation(
                    out=tt[:, jc * 512 :],
                    in_=mcs[s % 2][:, jc * 512 :],
                    func=mybir.ActivationFunctionType.Sqrt,
                    bias=bias_t[:, 0:1],
                    accum_out=res_t[:, s : s + 1],
                ).then_inc(act_sem, 1)
            # same-engine result DMA: no cross-engine hop; the self-wait
            # drains the last accum write before the DGE reads res_t
            scalar.wait_ge(act_sem, NSLOT)
            scalar.dma_start(out=res[:, :], in_=res_t[:, :]).then_inc(
                dma_out_sem, 16
            )

    return nc


def _augmented(xp32, xg32, w32):
    """U26 [B,N,26] (j side, w^2-scaled) and V26 [B,N,26] (i side)."""
    sp = np.sum(xp32 * xp32, axis=-1)  # [B,N]
    sg = np.sum(xg32 * xg32, axis=-1)
    ones = np.ones((B, N, 1), np.float32)
    up = np.concatenate([xp32, sp[..., None], ones], axis=-1)  # [B,N,5]
    up = up * (w32**2)[..., None]
    ug = np.concatenate([xg32, sg[..., None], ones], axis=-1)
    vp = np.concatenate([-2.0 * xp32, ones, sp[..., None]], axis=-1)
    vg = np.concatenate([-2.0 * xg32, ones, sg[..., None]], axis=-1)

    U = np.einsum("nja,njc->njac", up, ug).reshape(B, N, 25).astype(np.float32)
    V = np.einsum("nia,nic->niac", vp, vg).reshape(B, N, 25).astype(np.float32)
    z = np.zeros((B, N, 1), np.float32)
    return (
        np.concatenate([U, np.ones((B, N, 1), np.float32)], axis=-1),
        np.concatenate([V, z], axis=-1),
    )


def _mask_array(parity):
    """[128, 4*512] 0/1 mask; position k = variant (k + 2*parity) % 4.
    Variant v: keep j-chunk column q iff q > 128*v + p (strictly upper)."""
    q = np.arange(512)[None, :]
    p = np.arange(128)[:, None]
    parts = [
        (q > 128 * ((k + 2 * parity) % 4) + p).astype(np.float32) for k in range(4)
    ]
    return np.concatenate(parts, axis=1)


def _host_inputs(U26, V26):
    masks = {h: _mask_array(h) for h in (0, 1)}
    in_maps = []
    for core in range(8):
        b, h = core // 2, core % 2
        slots = SLOTS_BY_PARITY[h]
        vcols = np.concatenate(
            [U26[b].T] + [V26[b, bi * 128 : (bi + 1) * 128].T for bi in slots],
            axis=1,
        )
        in_maps.append({"uv": np.ascontiguousarray(vcols), "masks": masks[h]})
    return in_maps


def _host_assemble(xp32, xg32, ht32, w32, P):
    """Alignment loss + analytic bond parts + final scaling (f64)."""
    xp = xp32.astype(np.float64)
    xg = xg32.astype(np.float64)
    ht = ht32.astype(np.float64)
    w = w32.astype(np.float64)

    W = w.sum(axis=1)  # [B]
    # weighted_rigid_align(x_l=xGT, xGT_l=xpred, w): align GT onto pred frame
    mu = (w[..., None] * xg).sum(axis=1) / W[:, None]
    muGT = (w[..., None] * xp).sum(axis=1) / W[:, None]
    xc = xg - mu[:, None, :]
    xGTc = xp - muGT[:, None, :]
    M = np.einsum("bni,bnj->bij", w[..., None] * xGTc, xc)
    U, _, Vh = np.linalg.svd(M)
    R = U @ Vh
    det = np.linalg.det(R)
    Fm = np.diag([1.0, 1.0, -1.0])
    Rfix = np.einsum("bij,jk,bkl->bil", U, Fm, Vh)
    R = np.where(det[:, None, None] < 0, Rfix, R)
    xalign = np.einsum("bnj,bkj->bnk", xc, R) + muGT[:, None, :]
    lnum = (np.linalg.norm(xp - xalign, axis=-1) * w).sum()
    loss_align = lnum / W.sum()

    sp = (xp * xp).sum(-1)
    sg = (xg * xg).sum(-1)
    wxp = np.einsum("bn,bni->bi", w, xp)
    wxg = np.einsum("bn,bni->bi", w, xg)
    Ap = 2 * (W * (w * sp).sum(1) - (wxp**2).sum(1))
    Bg = 2 * (W * (w * sg).sum(1) - (wxg**2).sum(1))

    bond = (Ap + Bg - 2 * P).sum() / (W**2).sum()
    loss = loss_align + bond
    out = (ht**2 + SIGMA_DATA**2) / (ht + SIGMA_DATA) ** 2 * loss
    return out.astype(np.float32)


def kernel(xpred_l, xGT_l, ht, w_l):
    global _NC_CACHE
    xp32 = np.ascontiguousarray(np.asarray(xpred_l, dtype=np.float32))
    xg32 = np.ascontiguousarray(np.asarray(xGT_l, dtype=np.float32))
    ht32 = np.asarray(ht, dtype=np.float32)
    w32 = np.ascontiguousarray(np.asarray(w_l, dtype=np.float32))

    if _NC_CACHE is None:
        _NC_CACHE = _build_nc()
    nc = _NC_CACHE

    U26, V26 = _augmented(xp32, xg32, w32)
    in_maps = _host_inputs(U26, V26)
    results = run_bass_kernel_spmd(nc, in_maps, list(range(8))).results

    # res[p, s] = sum_{j>i} sqrt(max(w_j^2 dp^2 dg^2, 0)) for row
    # i = bi(s)*128 + p; apply w_i, double (symmetry), reduce in f64.
    P = np.zeros(B)
    for core in range(8):
        b, h = core // 2, core % 2
        r = results[core]["res"].astype(np.float64)
        for s, bi in enumerate(SLOTS_BY_PARITY[h]):
            wrow = w32[b, bi * 128 : (bi + 1) * 128].astype(np.float64)
            P[b] += 2.0 * (r[:, s] * wrow).sum()

    return _host_assemble(xp32, xg32, ht32, w32, P)
